# revision 2
# baseline (speedup 1.0000x reference)
"""ContrastiveLoss kernel for 8 Trainium2 NeuronCores (Bass/Tile).

Strategy (sharding hint): shard z by rows across 8 cores. Each core
normalizes + transposes its [1024, 1024] slab (PE transpose), casts to
fp8e4, AllGathers the normalized-transposed slabs (1MB -> 8.4MB), then
computes its [1024, 8192] slab of the cosine-similarity matrix with
fp8 DoubleRow matmuls (2 k-planes per call, 0.5 cycles/row), doing a
fused exp(x/T) + row-sum on the scalar engine (no max-subtraction
needed: logits are bounded by 1/T). The diagonal self-term is removed
by recomputing the local self-block with bit-identical fp8 matmuls and
subtracting its exp. Positives sim[i, (i-4096)%8192] are computed in
fp32 as row-wise dots of z_local with the positive slab (a host-sliced
input), so the instruction stream is identical on every core - only
data differs. Output: per-row NLL [128, 8] per core; host gathers and
takes the mean.
"""
import numpy as np

import concourse.bacc as bacc
from concourse import mybir
from concourse.tile import TileContext
from concourse.bass_utils import run_bass_kernel_spmd

N, D, C = 8192, 1024, 8
L = N // C            # rows per core
P = 128               # partitions
MT = L // P           # 8 row-tiles per core
KT = D // P           # 8 contraction chunks
KK = KT // 2          # 4 DoubleRow pair chunks
NB = 512              # matmul moving-dim tile
CB = N // NB          # 16 column blocks
TEMP = 0.07
SCALE = 1.0 / TEMP
EPS = 1e-8

F32 = mybir.dt.float32
DT = mybir.dt.float8e4  # matmul operand dtype (DoubleRow perf mode)
DR = mybir.MatmulPerfMode.DoubleRow

AF = mybir.ActivationFunctionType
ALU = mybir.AluOpType

_cached = {}


def _emit_pipeline(nc, z, zp, imf, lhs2, rnz, rnp, posd, Stiles, nll_sb,
                   ag_in, ag_out, zpool, rpool, epool, spool, pbig, psmall,
                   phases="ABCD"):
    # ---------------- Phase A: normalize + transpose local slab
    for m in range(MT):
        zt = zpool.tile([P, D], F32, tag="zt", name="zt")
        nc.sync.dma_start(out=zt[:, :], in_=z[m * P:(m + 1) * P, :])
        zpt = zpool.tile([P, D], F32, tag="zpt", name="zpt")
        nc.sync.dma_start(out=zpt[:, :], in_=zp[m * P:(m + 1) * P, :])

        ssq = spool.tile([P, 2], F32, tag="ssq", name="ssq")
        scr = zpool.tile([P, D], F32, tag="scr", name="scr")
        nc.scalar.activation(scr[:, :], zt[:, :], AF.Square,
                             accum_out=ssq[:, 0:1])
        scr2 = zpool.tile([P, D], F32, tag="scr2", name="scr2")
        nc.scalar.activation(scr2[:, :], zpt[:, :], AF.Square,
                             accum_out=ssq[:, 1:2])
        scr3 = zpool.tile([P, D], F32, tag="scr3", name="scr3")
        nc.vector.tensor_mul(scr3[:, :], zt[:, :], zpt[:, :])
        nc.vector.reduce_sum(posd[:, m:m + 1], scr3[:, :],
                             axis=mybir.AxisListType.X)

        nrm = spool.tile([P, 2], F32, tag="nrm", name="nrm")
        nc.scalar.activation(nrm[:, :], ssq[:, :], AF.Sqrt)
        nc.vector.tensor_scalar_max(nrm[:, :], nrm[:, :], EPS)
        rcp = spool.tile([P, 2], F32, tag="rcp", name="rcp")
        nc.vector.reciprocal(rcp[:, :], nrm[:, :])
        nc.vector.tensor_copy(rnz[:, m:m + 1], rcp[:, 0:1])
        nc.vector.tensor_copy(rnp[:, m:m + 1], rcp[:, 1:2])

        zn = zpool.tile([P, D], F32, tag="zn", name="zn")
        nc.scalar.activation(zn[:, :], zt[:, :], AF.Copy, scale=rcp[:, 0:1])
        for k in range(KT):
            pt = psmall.tile([P, P], F32, tag="small", name="pt")
            nc.tensor.transpose(pt[:, :], zn[:, k * P:(k + 1) * P], imf[:, :])
            nc.vector.tensor_copy(
                lhs2[k // 2][:, k % 2, m * P:(m + 1) * P], pt[:, :])

    # ---------------- Phase B: AllGather fp8 znT
    if "B" not in phases:
        return
    for kk in range(KK):
        nc.sync.dma_start(out=ag_in[kk, :, :, :], in_=lhs2[kk][:, :, :])
    nc.gpsimd.collective_compute(
        "AllGather", ALU.bypass,
        ins=[ag_in.ap().opt()],
        outs=[ag_out.ap().opt()],
        replica_groups=[list(range(C))],
    )

    # ---------------- Phase C: similarity slab + exp row-sums
    if "C" not in phases:
        return
    for cb in range(CB):
        r, h = cb // 2, cb % 2
        rts = []
        for kk in range(KK):
            rt = rpool.tile([P, 2, NB], DT, tag=f"rhs{kk}", name=f"rt{kk}")
            nc.sync.dma_start(
                out=rt[:, :, :],
                in_=ag_out[r, kk, :, :, h * NB:(h + 1) * NB])
            rts.append(rt)
        for m in range(MT):
            ps = pbig.tile([P, NB], F32, tag="big", name="ps")
            for kk in range(KK):
                nc.tensor.matmul(ps[:, :],
                                 lhs2[kk][:, :, m * P:(m + 1) * P],
                                 rts[kk][:, :, :],
                                 start=(kk == 0), stop=(kk == KK - 1),
                                 perf_mode=DR)
            esc = epool.tile([P, NB], F32, tag="esc", name="esc")
            nc.scalar.activation(esc[:, :], ps[:, :], AF.Exp, scale=SCALE,
                                 accum_out=Stiles[m][:, cb:cb + 1])

    # ---------------- Phase D: self-term removal + NLL
    if "D" not in phases:
        return
    for m in range(MT):
        pss = psmall.tile([P, P], F32, tag="small", name="pss")
        for kk in range(KK):
            nc.tensor.matmul(pss[:, :],
                             lhs2[kk][:, :, m * P:(m + 1) * P],
                             lhs2[kk][:, :, m * P:(m + 1) * P],
                             start=(kk == 0), stop=(kk == KK - 1),
                             perf_mode=DR)
        dscr = epool.tile([P, P], F32, tag="dscr", name="dscr")
        dv = spool.tile([P, 1], F32, tag="dv", name="dv")
        nc.vector.tensor_mul(dscr[:, :], pss[:, :], imf[:, :])
        nc.vector.reduce_sum(dv[:, 0:1], dscr[:, :],
                             axis=mybir.AxisListType.X)
        es = spool.tile([P, 1], F32, tag="es", name="es")
        nc.scalar.activation(es[:, 0:1], dv[:, 0:1], AF.Exp, scale=SCALE)
        sr = spool.tile([P, 1], F32, tag="sr", name="sr")
        nc.vector.reduce_sum(sr[:, 0:1], Stiles[m][:, :],
                             axis=mybir.AxisListType.X)
        sc = spool.tile([P, 1], F32, tag="sc", name="sc")
        nc.vector.tensor_sub(sc[:, 0:1], sr[:, 0:1], es[:, 0:1])
        lse = spool.tile([P, 1], F32, tag="lse", name="lse")
        nc.scalar.activation(lse[:, 0:1], sc[:, 0:1], AF.Ln)
        pr = spool.tile([P, 1], F32, tag="pr", name="pr")
        nc.vector.scalar_tensor_tensor(
            out=pr[:, 0:1], in0=posd[:, m:m + 1], scalar=rnz[:, m:m + 1],
            in1=rnp[:, m:m + 1], op0=ALU.mult, op1=ALU.mult)
        nc.vector.scalar_tensor_tensor(
            out=nll_sb[:, m:m + 1], in0=pr[:, 0:1], scalar=-SCALE,
            in1=lse[:, 0:1], op0=ALU.mult, op1=ALU.add)


def _build(reps: int = 1, phases: str = "ABCD"):
    nc = bacc.Bacc(trn_type="TRN2")
    z = nc.dram_tensor("z", [L, D], F32, kind="ExternalInput")
    zp = nc.dram_tensor("zp", [L, D], F32, kind="ExternalInput")
    im = nc.dram_tensor("im", [P, P], F32, kind="ExternalInput")
    nll_out = nc.dram_tensor("nll", [P, MT], F32, kind="ExternalOutput")

    ag_in = nc.dram_tensor("ag_in", [KK, P, 2, L], DT)
    ag_out = nc.dram_tensor("ag_out", [C, KK, P, 2, L], DT,
                            addr_space="Shared")

    with TileContext(nc) as tc:
        with (
            tc.tile_pool(name="const", bufs=1) as cpool,
            tc.tile_pool(name="lhs", bufs=1) as lpool,
            tc.tile_pool(name="stat", bufs=1) as spool,
            tc.tile_pool(name="prep", bufs=2) as zpool,
            tc.tile_pool(name="rhs", bufs=3) as rpool,
            tc.tile_pool(name="esc", bufs=3) as epool,
            tc.tile_pool(name="pbig", bufs=6, space="PSUM") as pbig,
            tc.tile_pool(name="psmall", bufs=2, space="PSUM") as psmall,
        ):
            imf = cpool.tile([P, P], F32, tag="imf")
            nc.sync.dma_start(out=imf[:, :], in_=im[:, :])

            lhs2 = [lpool.tile([P, 2, L], DT, tag=f"lhs{kk}", name=f"lhs{kk}")
                    for kk in range(KK)]
            rnz = spool.tile([P, MT], F32, tag="rnz")
            rnp = spool.tile([P, MT], F32, tag="rnp")
            posd = spool.tile([P, MT], F32, tag="posd")
            Stiles = [spool.tile([P, CB], F32, tag=f"S{m}", name=f"S{m}")
                      for m in range(MT)]
            nll_sb = spool.tile([P, MT], F32, tag="nll")

            for _rep in range(reps):
                _emit_pipeline(nc, z, zp, imf, lhs2, rnz, rnp, posd,
                               Stiles, nll_sb, ag_in, ag_out,
                               zpool, rpool, epool, spool, pbig, psmall,
                               phases=phases)

            nc.sync.dma_start(out=nll_out[:, :], in_=nll_sb[:, :])

    nc.finalize()
    return nc


def _build_repeat(reps: int, phases: str = "ABCD"):
    return _build(reps, phases)


def get_nc():
    if "nc" not in _cached:
        _cached["nc"] = _build()
    return _cached["nc"]


def kernel(z: np.ndarray, _profile: dict | None = None) -> np.ndarray:
    assert z.shape == (N, D)
    z = np.ascontiguousarray(z, dtype=np.float32)
    imask = np.eye(P, dtype=np.float32)
    in_maps = []
    for c in range(C):
        cp = (c + 4) % C
        in_maps.append({
            "z": z[c * L:(c + 1) * L],
            "zp": z[cp * L:(cp + 1) * L],
            "im": imask,
        })
    nc = get_nc()
    res = run_bass_kernel_spmd(nc, in_maps, core_ids=list(range(C)))
    if _profile is not None:
        _profile["exec_time_ns"] = res.exec_time_ns
        _profile["results"] = res
    # nll layout per core: [p, m] -> global row c*L + m*P + p
    total = 0.0
    for c in range(C):
        total += float(res.results[c]["nll"].sum(dtype=np.float64))
    return np.float32(total / N)
